# revision 18
# baseline (speedup 1.0000x reference)
"""Distributed MultiHeadAttention kernel for 8 Trainium2 NeuronCores.

Problem: B=2, L=2048, D=1024, H=16 heads (DH=64), causal attn_mask +
key_padding_mask, torch-Linear-convention projections.

Sharding: core = (batch b = core//4, group rank j = core%4). Each core
projects q/k/v for its batch restricted to its 4 heads (256 channels),
runs streaming softmax attention in a [key, query]-transposed layout
(scores are O(1) so no max subtraction; causal masking is structural
per 128-key block plus a -1e5 additive lower-triangle tile on diagonal
blocks). Key padding is folded into V: padded keys get zeroed V rows
AND a zeroed ones-column, so they contribute 0 to both numerator and
denominator -- the exp ACTIVATE needs no per-block bias at all.

v2 restructure vs the 304us baseline:
 - Both head-pairs are processed per query segment, then the segment's
   normalized attention (fp8) ships in ONE small AllGather chunk; 4
   chunked collectives pipeline under attention compute instead of two
   big bf16 AllGathers that exposed a ~57us serial tail.
 - o_proj is sharded by OUTPUT channel (each core computes out.T for
   its 256 output channels over all 2048 rows, host transposes), so
   every core consumes every AG chunk statically the moment it lands;
   o_proj matmuls are interleaved into attention-PE slack as "fillers"
   and replace the PE heater matmuls with real work.
 - Softmax division stays sender-side: S rides row 64 of the AV psum
   (ones column in V), 1/S via fast reciprocal on partition 64, then a
   K=1 matmul broadcasts it across the 64 channel partitions. The
   broadcast/ship half of the norm is deferred into the next segment's
   first block so its DVE dependencies never stall the PE stream.
 - Projection chunks are emitted as fillers inside attention blocks so
   the exp (ACT) stream starts ~12us in and the PE never idles long
   enough to re-throttle (HAM).

Matmuls run in bf16 (fp32 PE matmul is 4x slower); o_proj runs fp8
(weights + gathered activations). Inputs are transposed to [D, L] on
the host (DMA-transpose serializes on the xbar queue; host transpose
is free on the device timeline).
"""
import os
import sys

sys.path.insert(0, '/opt/trn_rl_repo')

import numpy as np
import ml_dtypes

import concourse.bass as bass
import concourse.bacc as bacc
import concourse.mybir as mybir
import concourse.tile as tile
from concourse.bass_utils import run_bass_kernel_spmd

BF16 = mybir.dt.bfloat16
F32 = mybir.dt.float32
FP8 = mybir.dt.float8e4
NPBF16 = ml_dtypes.bfloat16
NPFP8 = ml_dtypes.float8_e4m3

B, L, D, H = 2, 2048, 1024, 16
DH = D // H                      # 64
N_CORES = 8
GROUPS = [[0, 1, 2, 3], [4, 5, 6, 7]]
HPC = H // 4                     # heads per core = 4
CPC = HPC * DH                   # channels per core = 256
OPC = D // 4                     # output channels per core = 256
SEG = 512                        # query segment
NSEG = L // SEG                  # 4
KB = 128                         # key-block size
NKB = L // KB                    # 16
MASK_VAL = -1e5                  # causal: exp(MASK_VAL/8 + s) == 0
NDB = D // 128                   # 8 contraction blocks

ExpFn = mybir.ActivationFunctionType.Exp

_PROG_CACHE = {}
last_results = None


def _build_program():
    """Build the SPMD Bass program (identical on all 8 cores)."""
    nc = bacc.Bacc("TRN2", target_bir_lowering=False, debug=False,
                   num_devices=N_CORES)

    xqT = nc.declare_dram_parameter("xqT", [D, L], BF16, isOutput=False)
    xkT = nc.declare_dram_parameter("xkT", [D, L], BF16, isOutput=False)
    xvT = nc.declare_dram_parameter("xvT", [D, L], BF16, isOutput=False)
    wqT = nc.declare_dram_parameter("wqT", [D, CPC], BF16, isOutput=False)
    wkT = nc.declare_dram_parameter("wkT", [D, CPC], BF16, isOutput=False)
    wvT = nc.declare_dram_parameter("wvT", [D, CPC], BF16, isOutput=False)
    wo_in = nc.declare_dram_parameter("woT", [D, OPC], BF16, isOutput=False)
    bq_in = nc.declare_dram_parameter("bq", [128, 2], F32, isOutput=False)
    bk_in = nc.declare_dram_parameter("bk", [128, 2], F32, isOutput=False)
    bv_in = nc.declare_dram_parameter("bv", [1, CPC], BF16, isOutput=False)
    bo_in = nc.declare_dram_parameter("bo2", [128, 2], F32, isOutput=False)
    pad_in = nc.declare_dram_parameter("pad01", [128, NKB], F32,
                                       isOutput=False)
    vones_in = nc.declare_dram_parameter("vones", [128, NKB, HPC, 1], BF16,
                                         isOutput=False)
    tri_in = nc.declare_dram_parameter("tri", [128, 128], F32, isOutput=False)
    out = nc.declare_dram_parameter("out", [OPC, L], F32, isOutput=True)

    # per-segment AllGather bounce buffers (fp8 normalized attention):
    # chunk s carries [128ch, pair, 512 q] for query segment s.
    ag_in = [nc.dram_tensor(f"ag_in{s}", [128, 2, SEG], BF16)
             for s in range(NSEG)]
    ag_out = [nc.dram_tensor(f"ag_out{s}", [4, 128, 2, SEG], BF16)
              for s in range(NSEG)]

    with tile.TileContext(nc, num_cores=N_CORES) as tc:
        with tc.tile_pool(name="persist", bufs=1) as pers:
            wq_sb = pers.tile([128, NDB, CPC], BF16, tag="wq")
            wk_sb = pers.tile([128, NDB, CPC], BF16, tag="wk")
            wv_sb = pers.tile([128, NDB, CPC], BF16, tag="wv")
            wo_sb = pers.tile([128, NDB, OPC], BF16, tag="wo")
            bq_sb = pers.tile([128, 2], F32, tag="bq")
            bk_sb = pers.tile([128, 2], F32, tag="bk")
            bv_sb = pers.tile([1, CPC], BF16, tag="bv")
            bo_sb = pers.tile([128, 2], F32, tag="bo")
            pad_sb = pers.tile([128, NKB], F32, tag="pad")
            tri_sb = pers.tile([128, 128], F32, tag="tri")
            ones_sb = pers.tile([1, 128], BF16, tag="ones")
            onesb_sb = pers.tile([128, 64], BF16, tag="onesb")
            qT_sb = pers.tile([128, 2, L], BF16, tag="qT")
            kT_sb = pers.tile([128, 2, L], BF16, tag="kT")
            v_sb = pers.tile([128, NKB, HPC, DH + 1], BF16, tag="v")
            recb0_sb = pers.tile([32, 2, 2, SEG], BF16, tag="recb0")
            dumm_sb = pers.tile([1, 1], BF16, tag="dumm")

            # spread big loads across three DMA queues; small constants go
            # on the (idle-early) gpsimd queue so they never delay x-chunks
            nc.sync.dma_start(
                out=wq_sb[:], in_=wqT.ap().rearrange("(db p) c -> p db c", p=128))
            nc.scalar.dma_start(
                out=wk_sb[:], in_=wkT.ap().rearrange("(db p) c -> p db c", p=128))
            nc.gpsimd.dma_start(
                out=wv_sb[:], in_=wvT.ap().rearrange("(db p) c -> p db c", p=128))
            nc.gpsimd.dma_start(out=bq_sb[:], in_=bq_in[:])
            nc.gpsimd.dma_start(out=bk_sb[:], in_=bk_in[:])
            nc.gpsimd.dma_start(out=bv_sb[:], in_=bv_in[:])
            nc.gpsimd.dma_start(out=bo_sb[:], in_=bo_in[:])
            nc.gpsimd.dma_start(out=pad_sb[:], in_=pad_in[:])
            nc.gpsimd.dma_start(out=tri_sb[:], in_=tri_in[:])
            nc.gpsimd.dma_start(out=v_sb[:, :, :, DH:DH + 1], in_=vones_in[:])
            nc.scalar.dma_start(
                out=wo_sb[:], in_=wo_in.ap().rearrange("(db p) c -> p db c", p=128))
            nc.vector.memset(ones_sb[:], 1.0)
            # row 0 = 1, rows 1-31 = 0: the broadcast matmul runs K=32
            # from partition 0 because HW contracts the full 32-row group
            # (stale weights x garbage otherwise; the sim won't show this)
            nc.vector.memset(onesb_sb[:], 0.0)
            nc.vector.memset(onesb_sb[0:1, :], 1.0)
            # 1/S staging rows; recb0 rows 1-31 stay zero forever so the
            # K=32 broadcast contracts zeros beyond row 0
            nc.vector.memset(recb0_sb[:], 0.0)
            # preload the exp table set during the input DMA wait
            nc.vector.memset(dumm_sb[:], 0.0)
            nc.scalar.activation(out=dumm_sb[:], in_=dumm_sb[:], func=ExpFn)
            # PE heater: warm the HAM clock gate while input DMAs stream
            heat_sb = pers.tile([128, 1024], BF16, tag="heat")
            nc.vector.memset(heat_sb[:], 0.001)
            with tc.tile_pool(name="psH", bufs=1, space="PSUM") as psH:
                hps = psH.tile([128, 512], F32, tag="hps")
                for it in range(32):
                    nc.tensor.matmul(hps[:], lhsT=heat_sb[:, 0:128],
                                     rhs=heat_sb[:, 512:1024],
                                     start=(it == 0), stop=(it == 31))

            ctxPA = nc.named_scope("phasePA"); ctxPA.__enter__()
            with tc.tile_pool(name="xt", bufs=2) as xtp, \
                 tc.tile_pool(name="ps1", bufs=2, space="PSUM") as ps1p, \
                 tc.tile_pool(name="ex", bufs=3) as exp_pool, \
                 tc.tile_pool(name="sm", bufs=2) as smalls, \
                 tc.tile_pool(name="fnp", bufs=2) as fnp, \
                 tc.tile_pool(name="fatp", bufs=2) as fatp, \
                 tc.tile_pool(name="obp", bufs=2) as obp, \
                 tc.tile_pool(name="psX", bufs=2, space="PSUM") as psX, \
                 tc.tile_pool(name="psA", bufs=2, space="PSUM") as psA:

                xts = {}

                def emit_x_dma(lc):
                    l0 = lc * SEG
                    xtq = xtp.tile([128, NDB, SEG], BF16, tag="xtq",
                                   name=f"xtq_{lc}")
                    xtk = xtp.tile([128, NDB, SEG], BF16, tag="xtk",
                                   name=f"xtk_{lc}")
                    xtv = xtp.tile([128, NDB, SEG], BF16, tag="xtv",
                                   name=f"xtv_{lc}")
                    nc.sync.dma_start(
                        out=xtq[:],
                        in_=xqT.ap().rearrange("(db p) l -> p db l", p=128)
                        [:, :, l0:l0 + SEG])
                    nc.scalar.dma_start(
                        out=xtk[:],
                        in_=xkT.ap().rearrange("(db p) l -> p db l", p=128)
                        [:, :, l0:l0 + SEG])
                    nc.gpsimd.dma_start(
                        out=xtv[:],
                        in_=xvT.ap().rearrange("(db p) l -> p db l", p=128)
                        [:, :, l0:l0 + SEG])
                    xts[lc] = (xtq, xtk, xtv)

                def qk_fillers(lc):
                    """8 half-group fillers projecting q,k for chunk lc.
                    All tile allocations happen inside the closures so pool
                    buffer rotation matches emission order exactly."""
                    l0 = lc * SEG
                    fillers = []
                    for (w_sb, b_sb, t_sb, xi) in ((wq_sb, bq_sb, qT_sb, 0),
                                                   (wk_sb, bk_sb, kT_sb, 1)):
                        cell = {}

                        def h1(cell=cell, w_sb=w_sb, xi=xi, lc=lc):
                            cb = cell["cb"] = cell.get("cb", -1) + 1
                            ps = cell[cb] = ps1p.tile(
                                [128, SEG], F32, tag="ps1",
                                name=f"psqk_{lc}_{xi}_{cb}")
                            x_sb = xts[lc][xi]
                            for db in range(4):
                                nc.tensor.matmul(
                                    ps[:],
                                    lhsT=w_sb[:, db, cb * 128:(cb + 1) * 128],
                                    rhs=x_sb[:, db, :],
                                    start=(db == 0), stop=False)

                        def h2(cell=cell, w_sb=w_sb, b_sb=b_sb, t_sb=t_sb,
                               xi=xi, lc=lc, l0=l0):
                            cb = cell["cb"]
                            ps = cell[cb]
                            x_sb = xts[lc][xi]
                            for db in range(4, NDB):
                                nc.tensor.matmul(
                                    ps[:],
                                    lhsT=w_sb[:, db, cb * 128:(cb + 1) * 128],
                                    rhs=x_sb[:, db, :],
                                    start=False, stop=(db == NDB - 1))
                            nc.vector.tensor_scalar_add(
                                t_sb[:, cb, l0:l0 + SEG], ps[:],
                                b_sb[:, cb:cb + 1])
                        fillers += [h1, h2]
                    # order: q cb0, q cb1, k cb0, k cb1 -> rearrange so both
                    # pair-0 groups (q cb0, k cb0) come first
                    q1, q2, k1, k2 = fillers
                    return [q1, q2, k1, k2, q1, q2, k1, k2]

                def v_fillers(lc):
                    """8 half-group fillers projecting v for chunk lc."""
                    fillers = []
                    for ls in range(4):
                        kbg = lc * 4 + ls
                        cell = {}

                        def h1(cell=cell, ls=ls, lc=lc):
                            psv = cell["psv"] = ps1p.tile(
                                [128, CPC], F32, tag="ps1",
                                name=f"psv_{lc}_{ls}")
                            xtv = xts[lc][2]
                            for db in range(4):
                                nc.tensor.matmul(
                                    psv[:],
                                    lhsT=xtv[:, db, ls * 128:(ls + 1) * 128],
                                    rhs=wv_sb[:, db, :],
                                    start=(db == 0), stop=False)

                        def h2(cell=cell, ls=ls, lc=lc, kbg=kbg):
                            psv = cell["psv"]
                            xtv = xts[lc][2]
                            for db in range(4, NDB):
                                nc.tensor.matmul(
                                    psv[:],
                                    lhsT=xtv[:, db, ls * 128:(ls + 1) * 128],
                                    rhs=wv_sb[:, db, :],
                                    start=False, stop=False)
                            nc.tensor.matmul(
                                psv[:], lhsT=ones_sb[:, 0:128], rhs=bv_sb[:],
                                start=False, stop=True)
                            # padded keys: zero both V rows and the ones col
                            nc.vector.tensor_scalar_mul(
                                v_sb[:, kbg, :, 0:DH],
                                psv[:].rearrange("p (h d) -> p h d", h=HPC),
                                pad_sb[:, kbg:kbg + 1])
                        fillers += [h1, h2]
                    return fillers

                def o_fillers(s):
                    """5 fillers: consume AG chunk s -> out.T rows."""
                    cell = {}

                    def f_dma(cell=cell, s=s):
                        fat = cell["fat"] = fatp.tile(
                            [128, 4, 2, SEG], BF16, tag="fat", name=f"fat_{s}")
                        nc.scalar.dma_start(
                            out=fat[:],
                            in_=ag_out[s].ap().rearrange("r c p l -> c r p l"))
                    fillers = [f_dma]
                    for ob in range(2):
                        def f1(cell=cell, ob=ob, s=s):
                            po = cell[ob] = ps1p.tile(
                                [128, SEG], F32, tag="ps1", name=f"po_{s}_{ob}")
                            fat = cell["fat"]
                            for i, (p, r) in enumerate(
                                    (p, r) for r in range(2) for p in range(2)):
                                nc.tensor.matmul(
                                    po[:],
                                    lhsT=wo_sb[:, 2 * r + p,
                                               ob * 128:(ob + 1) * 128],
                                    rhs=fat[:, r, p, :],
                                    start=(i == 0), stop=False)

                        def f2(cell=cell, ob=ob, s=s):
                            po = cell[ob]
                            fat = cell["fat"]
                            for i, (p, r) in enumerate(
                                    (p, r) for r in range(2, 4) for p in range(2)):
                                nc.tensor.matmul(
                                    po[:],
                                    lhsT=wo_sb[:, 2 * r + p,
                                               ob * 128:(ob + 1) * 128],
                                    rhs=fat[:, r, p, :],
                                    start=False, stop=(i == 3))
                            ob_sb = obp.tile([128, SEG], F32, tag="ob",
                                             name=f"ob_{s}_{ob}")
                            nc.vector.tensor_scalar_add(
                                ob_sb[:], po[:], bo_sb[:, ob:ob + 1])
                            nc.sync.dma_start(
                                out=out[ob * 128:(ob + 1) * 128,
                                        s * SEG:(s + 1) * SEG],
                                in_=ob_sb[:])
                        fillers += [f1, f2]
                    return fillers

                def emit_A_seg(p, s, fillers=(), pre=()):
                    """Attention for pair p, query segment s; returns pa.

                    pre: closures emitted after block 0/1's exp, BEFORE this
                    segment's pa allocation (deferred norm of the previous
                    segment + its AllGather)."""
                    fillers = list(fillers)
                    pre = list(pre)
                    nkb = (s + 1) * 4
                    pa = None
                    for kb in range(nkb):
                        o = max(0, kb * KB - s * SEG)
                        ps = psX.tile([128, 2, SEG], F32, tag="psX",
                                      name=f"ps_{p}_{s}_{kb}")
                        for hp in range(2):
                            hoff = hp * 64
                            # both heads of the pair target different
                            # row groups + PSUM banks -> concurrent MMs
                            nc.tensor.matmul(
                                ps[:, hp, o:SEG],
                                lhsT=kT_sb[hoff:hoff + 64, p,
                                           kb * KB:(kb + 1) * KB],
                                rhs=qT_sb[hoff:hoff + 64, p,
                                          s * SEG + o:(s + 1) * SEG],
                                start=True, stop=True)
                        if kb >= s * 4:  # diagonal block: causal tri
                            for hp in range(2):
                                nc.vector.tensor_add(
                                    ps[:, hp, o:o + 128],
                                    ps[:, hp, o:o + 128], tri_sb[:])
                        ex = exp_pool.tile([128, 2, SEG], BF16, tag="ex",
                                           name=f"ex_{p}_{s}_{kb}")
                        nc.scalar.activation(
                            out=ex[:, :, o:], in_=ps[:, :, o:], func=ExpFn,
                            scale=0.125)
                        if pre:
                            pre.pop(0)()
                        if pa is None:
                            pa = {hp: psA.tile([65, SEG], F32, tag="pa",
                                               name=f"pa_{p}_{s}_{hp}")
                                  for hp in range(2)}
                        for hp in range(2):
                            h = p * 2 + hp
                            nc.tensor.matmul(
                                pa[hp][:, o:], lhsT=v_sb[:, kb, h, :],
                                rhs=ex[:, hp, o:],
                                start=(kb == 0), stop=(kb == nkb - 1))
                        # drain fillers; finish by end of block nkb-2 so
                        # v-projection evacs precede their diagonal blocks
                        nleft = max(1, nkb - 1 - kb)
                        npop = -(-len(fillers) // nleft) if fillers else 0
                        for _ in range(npop):
                            fillers.pop(0)()
                    return pa

                def norm_a(p, s, pa):
                    """Copy S rows off psum, DMA-hop them to partition 0
                    (engines cannot move data across partitions, and the
                    custom reciprocal op only works from partition 0)."""
                    s1 = smalls.tile([65, 2, SEG], F32, tag="s1",
                                     name=f"s1_{p}_{s}")
                    sraw = smalls.tile([1, 2 * SEG], F32, tag="sraw",
                                       name=f"sraw_{p}_{s}")
                    for hp in range(2):
                        nc.vector.tensor_copy(s1[64:65, hp, :],
                                              pa[hp][64:65, :])
                    nc.gpsimd.dma_start(out=sraw[0:1, :],
                                        in_=s1[64:65, :, :])
                    return sraw

                def norm_b(p, s, pa, sraw):
                    """Reciprocal at partition 0, broadcast via one-hot
                    matmul, multiply, ship to the AG bounce."""
                    def run():
                        srec = smalls.tile([1, 2 * SEG], F32, tag="srec",
                                           name=f"srec_{p}_{s}")
                        nc.vector.reciprocal_approx_fast(srec[:], sraw[:])
                        nc.vector.tensor_copy(
                            recb0_sb[0:1, p, :, :],
                            srec[:].rearrange("p (hp n) -> p hp n", hp=2))
                        # K=32 zero-padded one-hot broadcasts 1/S to the 64
                        # channel partitions
                        bcps = psX.tile([128, 2, SEG], F32, tag="psX",
                                        name=f"bc_{p}_{s}")
                        for hp in range(2):
                            nc.tensor.matmul(
                                bcps[0:64, hp, :],
                                lhsT=onesb_sb[0:32, 0:64],
                                rhs=recb0_sb[0:32, p, hp, :],
                                start=True, stop=True)
                        bcs = smalls.tile([64, 2, SEG], F32, tag="bcs",
                                          name=f"bcs_{p}_{s}")
                        nc.vector.tensor_copy(bcs[:], bcps[0:64, :, :])
                        fn = fnp.tile([64, 2, SEG], BF16, tag="fn",
                                      name=f"fn_{p}_{s}")
                        for hp in range(2):
                            nc.vector.tensor_mul(
                                fn[:, hp, :], pa[hp][0:64, :], bcs[:, hp, :])
                        nc.gpsimd.dma_start(
                            out=ag_in[s].ap()[:, p, :].rearrange(
                                "(hp c) l -> c hp l", hp=2),
                            in_=fn[:])
                    return run

                def emit_ag(s):
                    def run():
                        nc.gpsimd.collective_compute(
                            "AllGather", mybir.AluOpType.bypass,
                            replica_groups=GROUPS,
                            ins=[ag_in[s][:]], outs=[ag_out[s][:]])
                    return run

                # ---------------- emission schedule ----------------
                emit_x_dma(0)
                emit_x_dma(1)
                # chunk 0 projected directly (gates the first exp)
                qk0 = qk_fillers(0)
                vf0 = v_fillers(0)
                for f in qk0[:4]:             # q cb0, k cb0
                    f()
                for f in vf0[:4]:             # v kbg 0,1
                    f()
                pa00 = emit_A_seg(0, 0, vf0[4:])
                s100 = norm_a(0, 0, pa00)
                for f in qk0[4:]:             # q cb1, k cb1
                    f()
                pa10 = emit_A_seg(1, 0, qk_fillers(1),
                                  pre=[norm_b(0, 0, pa00, s100)])
                s110 = norm_a(1, 0, pa10)

                pa01 = emit_A_seg(0, 1, v_fillers(1),
                                  pre=[norm_b(1, 0, pa10, s110), emit_ag(0)])
                s101 = norm_a(0, 1, pa01)
                emit_x_dma(2)
                pa11 = emit_A_seg(1, 1, qk_fillers(2),
                                  pre=[norm_b(0, 1, pa01, s101)])
                s111 = norm_a(1, 1, pa11)

                pa02 = emit_A_seg(0, 2, v_fillers(2) + o_fillers(0),
                                  pre=[norm_b(1, 1, pa11, s111), emit_ag(1)])
                s102 = norm_a(0, 2, pa02)
                emit_x_dma(3)
                pa12 = emit_A_seg(1, 2, qk_fillers(3),
                                  pre=[norm_b(0, 2, pa02, s102)])
                s112 = norm_a(1, 2, pa12)

                pa03 = emit_A_seg(0, 3, v_fillers(3) + o_fillers(1),
                                  pre=[norm_b(1, 2, pa12, s112), emit_ag(2)])
                s103 = norm_a(0, 3, pa03)
                pa13 = emit_A_seg(1, 3, o_fillers(2),
                                  pre=[norm_b(0, 3, pa03, s103)])
                s113 = norm_a(1, 3, pa13)
                norm_b(1, 3, pa13, s113)()
                emit_ag(3)()

                # keep the PE clock warm while AG(3) drains
                hps2 = ps1p.tile([128, 128], F32, tag="ps1", name="hps2")
                for it in range(48):
                    nc.tensor.matmul(hps2[:], lhsT=heat_sb[:, 0:128],
                                     rhs=heat_sb[:, 512:640],
                                     start=(it == 0), stop=(it == 47))
                for f in o_fillers(3):
                    f()

            ctxPA.__exit__(None, None, None)
    nc.compile()
    return nc


def _check_masks(attn_mask, key_padding_mask):
    """The fast path handles exactly-causal attn_mask plus any key padding
    that leaves key 0 unpadded (no all-masked softmax rows)."""
    causal = np.triu(np.ones((L, L), np.bool_), k=1)
    if not np.array_equal(attn_mask, causal):
        return None, True
    if key_padding_mask[:, 0].any():
        return None, True
    pad = [np.ascontiguousarray(
        np.where(key_padding_mask[b].reshape(NKB, KB).T,
                 np.float32(0.0), np.float32(1.0)))
           for b in range(B)]                              # [128, NKB]
    return pad, False


def _host_fallback(query, key, value, attn_mask, key_padding_mask,
                   Wq, bq, Wk, bk, Wv, bv, Wo, bo):
    """Exact fp32 numpy replica of the reference (degenerate masks only)."""
    q = (query @ Wq.T + bq).reshape(B, L, H, DH).transpose(0, 2, 1, 3)
    k = (key @ Wk.T + bk).reshape(B, L, H, DH).transpose(0, 2, 1, 3)
    v = (value @ Wv.T + bv).reshape(B, L, H, DH).transpose(0, 2, 1, 3)
    scores = np.einsum('bhqd,bhkd->bhqk', q, k) / np.sqrt(np.float32(DH))
    scores = np.where(key_padding_mask[:, None, None, :], -1e30, scores)
    scores = np.where(attn_mask[None, None, :, :], -1e30, scores)
    scores = scores - scores.max(axis=-1, keepdims=True)
    w = np.exp(scores)
    w = w / w.sum(axis=-1, keepdims=True)
    attn = np.einsum('bhqk,bhkd->bhqd', w, v)
    attn = attn.transpose(0, 2, 1, 3).reshape(B, L, D)
    return (attn @ Wo.T + bo).astype(np.float32)


def kernel(query, key, value, attn_mask, key_padding_mask,
           Wq, bq, Wk, bk, Wv, bv, Wo, bo):
    global last_results
    query = np.asarray(query, dtype=np.float32)
    key = np.asarray(key, dtype=np.float32)
    value = np.asarray(value, dtype=np.float32)
    attn_mask = np.asarray(attn_mask, dtype=bool)
    key_padding_mask = np.asarray(key_padding_mask, dtype=bool)
    Wq, bq = np.asarray(Wq, np.float32), np.asarray(bq, np.float32)
    Wk, bk = np.asarray(Wk, np.float32), np.asarray(bk, np.float32)
    Wv, bv = np.asarray(Wv, np.float32), np.asarray(bv, np.float32)
    Wo, bo = np.asarray(Wo, np.float32), np.asarray(bo, np.float32)

    pad_bufs, degenerate = _check_masks(attn_mask, key_padding_mask)
    if degenerate:
        return _host_fallback(query, key, value, attn_mask, key_padding_mask,
                              Wq, bq, Wk, bk, Wv, bv, Wo, bo)

    if "prog" not in _PROG_CACHE:
        _PROG_CACHE["prog"] = _build_program()
    nc = _PROG_CACHE["prog"]

    tri = np.where(np.arange(128)[None, :] < np.arange(128)[:, None],
                   np.float32(MASK_VAL), np.float32(0.0))   # [k', q']
    xT_bf = [np.ascontiguousarray(a.transpose(0, 2, 1)).astype(NPBF16)
             for a in (query, key, value)]             # [B, D, L] bf16

    in_maps = []
    for core in range(N_CORES):
        b, j = divmod(core, 4)
        csl = slice(j * CPC, (j + 1) * CPC)
        osl = slice(j * OPC, (j + 1) * OPC)
        vones = np.ascontiguousarray(
            pad_bufs[b][:, :, None, None].repeat(HPC, axis=2)).astype(NPBF16)
        in_maps.append({
            "xqT": xT_bf[0][b],
            "xkT": xT_bf[1][b],
            "xvT": xT_bf[2][b],
            "wqT": np.ascontiguousarray(Wq[csl, :].T).astype(NPBF16),
            "wkT": np.ascontiguousarray(Wk[csl, :].T).astype(NPBF16),
            "wvT": np.ascontiguousarray(Wv[csl, :].T).astype(NPBF16),
            "woT": np.ascontiguousarray(Wo.T[:, osl]).astype(NPBF16),
            "bq": np.ascontiguousarray(bq[csl].reshape(2, 128).T),
            "bk": np.ascontiguousarray(bk[csl].reshape(2, 128).T),
            "bv": bv[csl].reshape(1, CPC).astype(NPBF16),
            "bo2": np.ascontiguousarray(bo[osl].reshape(2, 128).T),
            "pad01": pad_bufs[b],
            "vones": vones,
            "tri": tri,
        })

    trace = os.environ.get("KERNEL_TRACE", "0") == "1"
    res = run_bass_kernel_spmd(nc, in_maps, list(range(N_CORES)), trace=trace)
    last_results = res

    outf = np.empty((B, L, D), dtype=np.float32)
    for core in range(N_CORES):
        b, j = divmod(core, 4)
        outf[b, :, j * OPC:(j + 1) * OPC] = res.results[core]["out"].T
    return outf


# revision 19
# speedup vs baseline: 1.0345x; 1.0345x over previous
"""Distributed MultiHeadAttention kernel for 8 Trainium2 NeuronCores.

Problem: B=2, L=2048, D=1024, H=16 heads (DH=64), causal attn_mask +
key_padding_mask, torch-Linear-convention projections.

Sharding: core = (batch b = core//4, group rank j = core%4). Each core
projects q/k/v for its batch restricted to its 4 heads (256 channels),
runs streaming softmax attention in a [key, query]-transposed layout
(scores are O(1) so no max subtraction; causal masking is structural
per 128-key block plus a -1e5 additive lower-triangle tile on diagonal
blocks). Key padding is folded into V: padded keys get zeroed V rows
AND a zeroed ones-column, so they contribute 0 to both numerator and
denominator -- the exp ACTIVATE needs no per-block bias at all.

v2 restructure vs the 304us baseline:
 - Both head-pairs are processed per query segment, then the segment's
   normalized attention (fp8) ships in ONE small AllGather chunk; 4
   chunked collectives pipeline under attention compute instead of two
   big bf16 AllGathers that exposed a ~57us serial tail.
 - o_proj is sharded by OUTPUT channel (each core computes out.T for
   its 256 output channels over all 2048 rows, host transposes), so
   every core consumes every AG chunk statically the moment it lands;
   o_proj matmuls are interleaved into attention-PE slack as "fillers"
   and replace the PE heater matmuls with real work.
 - Softmax division stays sender-side: S rides row 64 of the AV psum
   (ones column in V), 1/S via fast reciprocal on partition 64, then a
   K=1 matmul broadcasts it across the 64 channel partitions. The
   broadcast/ship half of the norm is deferred into the next segment's
   first block so its DVE dependencies never stall the PE stream.
 - Projection chunks are emitted as fillers inside attention blocks so
   the exp (ACT) stream starts ~12us in and the PE never idles long
   enough to re-throttle (HAM).

Matmuls run in bf16 (fp32 PE matmul is 4x slower); o_proj runs fp8
(weights + gathered activations). Inputs are transposed to [D, L] on
the host (DMA-transpose serializes on the xbar queue; host transpose
is free on the device timeline).
"""
import os
import sys

sys.path.insert(0, '/opt/trn_rl_repo')

import numpy as np
import ml_dtypes

import concourse.bass as bass
import concourse.bacc as bacc
import concourse.mybir as mybir
import concourse.tile as tile
from concourse.bass_utils import run_bass_kernel_spmd

BF16 = mybir.dt.bfloat16
F32 = mybir.dt.float32
FP8 = mybir.dt.float8e4
NPBF16 = ml_dtypes.bfloat16
NPFP8 = ml_dtypes.float8_e4m3

B, L, D, H = 2, 2048, 1024, 16
DH = D // H                      # 64
N_CORES = 8
GROUPS = [[0, 1, 2, 3], [4, 5, 6, 7]]
HPC = H // 4                     # heads per core = 4
CPC = HPC * DH                   # channels per core = 256
OPC = D // 4                     # output channels per core = 256
SEG = 512                        # query segment
NSEG = L // SEG                  # 4
KB = 128                         # key-block size
NKB = L // KB                    # 16
MASK_VAL = -1e5                  # causal: exp(MASK_VAL/8 + s) == 0
NDB = D // 128                   # 8 contraction blocks

ExpFn = mybir.ActivationFunctionType.Exp

_PROG_CACHE = {}
last_results = None


def _build_program():
    """Build the SPMD Bass program (identical on all 8 cores)."""
    nc = bacc.Bacc("TRN2", target_bir_lowering=False, debug=False,
                   num_devices=N_CORES)

    xqT = nc.declare_dram_parameter("xqT", [D, L], BF16, isOutput=False)
    xkT = nc.declare_dram_parameter("xkT", [D, L], BF16, isOutput=False)
    xvT = nc.declare_dram_parameter("xvT", [D, L], BF16, isOutput=False)
    wqT = nc.declare_dram_parameter("wqT", [D, CPC], BF16, isOutput=False)
    wkT = nc.declare_dram_parameter("wkT", [D, CPC], BF16, isOutput=False)
    wvT = nc.declare_dram_parameter("wvT", [D, CPC], BF16, isOutput=False)
    wo_in = nc.declare_dram_parameter("woT", [D, OPC], BF16, isOutput=False)
    bq_in = nc.declare_dram_parameter("bq", [128, 2], F32, isOutput=False)
    bk_in = nc.declare_dram_parameter("bk", [128, 2], F32, isOutput=False)
    bv_in = nc.declare_dram_parameter("bv", [1, CPC], BF16, isOutput=False)
    bo_in = nc.declare_dram_parameter("bo2", [128, 2], F32, isOutput=False)
    pad_in = nc.declare_dram_parameter("pad01", [128, NKB], F32,
                                       isOutput=False)
    vones_in = nc.declare_dram_parameter("vones", [128, NKB, HPC, 1], BF16,
                                         isOutput=False)
    tri_in = nc.declare_dram_parameter("tri", [128, 128], F32, isOutput=False)
    out = nc.declare_dram_parameter("out", [OPC, L], F32, isOutput=True)

    dag_in = nc.dram_tensor("dag_in", [1, 128], BF16)
    dag_out = nc.dram_tensor("dag_out", [4, 1, 128], BF16)
    # per-segment AllGather bounce buffers (fp8 normalized attention):
    # chunk s carries [128ch, pair, 512 q] for query segment s.
    ag_in = [nc.dram_tensor(f"ag_in{s}", [128, 2, SEG], BF16)
             for s in range(NSEG)]
    ag_out = [nc.dram_tensor(f"ag_out{s}", [4, 128, 2, SEG], BF16)
              for s in range(NSEG)]

    with tile.TileContext(nc, num_cores=N_CORES) as tc:
        with tc.tile_pool(name="persist", bufs=1) as pers:
            wq_sb = pers.tile([128, NDB, CPC], BF16, tag="wq")
            wk_sb = pers.tile([128, NDB, CPC], BF16, tag="wk")
            wv_sb = pers.tile([128, NDB, CPC], BF16, tag="wv")
            wo_sb = pers.tile([128, NDB, OPC], BF16, tag="wo")
            bq_sb = pers.tile([128, 2], F32, tag="bq")
            bk_sb = pers.tile([128, 2], F32, tag="bk")
            bv_sb = pers.tile([1, CPC], BF16, tag="bv")
            bo_sb = pers.tile([128, 2], F32, tag="bo")
            pad_sb = pers.tile([128, NKB], F32, tag="pad")
            tri_sb = pers.tile([128, 128], F32, tag="tri")
            ones_sb = pers.tile([1, 128], BF16, tag="ones")
            onesb_sb = pers.tile([128, 64], BF16, tag="onesb")
            qT_sb = pers.tile([128, 2, L], BF16, tag="qT")
            kT_sb = pers.tile([128, 2, L], BF16, tag="kT")
            v_sb = pers.tile([128, NKB, HPC, DH + 1], BF16, tag="v")
            recb0_sb = pers.tile([32, 2, 2, SEG], BF16, tag="recb0")
            dumm_sb = pers.tile([1, 1], BF16, tag="dumm")

            # spread big loads across three DMA queues; small constants go
            # on the (idle-early) gpsimd queue so they never delay x-chunks
            nc.sync.dma_start(
                out=wq_sb[:], in_=wqT.ap().rearrange("(db p) c -> p db c", p=128))
            nc.scalar.dma_start(
                out=wk_sb[:], in_=wkT.ap().rearrange("(db p) c -> p db c", p=128))
            nc.scalar.dma_start(
                out=wv_sb[:], in_=wvT.ap().rearrange("(db p) c -> p db c", p=128))
            nc.gpsimd.dma_start(out=bq_sb[:], in_=bq_in[:])
            nc.gpsimd.dma_start(out=bk_sb[:], in_=bk_in[:])
            nc.gpsimd.dma_start(out=bv_sb[:], in_=bv_in[:])
            nc.gpsimd.dma_start(out=bo_sb[:], in_=bo_in[:])
            nc.gpsimd.dma_start(out=pad_sb[:], in_=pad_in[:])
            nc.gpsimd.dma_start(out=tri_sb[:], in_=tri_in[:])
            nc.gpsimd.dma_start(out=v_sb[:, :, :, DH:DH + 1], in_=vones_in[:])
            nc.vector.memset(ones_sb[:], 1.0)
            # row 0 = 1, rows 1-31 = 0: the broadcast matmul runs K=32
            # from partition 0 because HW contracts the full 32-row group
            # (stale weights x garbage otherwise; the sim won't show this)
            nc.vector.memset(onesb_sb[:], 0.0)
            nc.vector.memset(onesb_sb[0:1, :], 1.0)
            # 1/S staging rows; recb0 rows 1-31 stay zero forever so the
            # K=32 broadcast contracts zeros beyond row 0
            nc.vector.memset(recb0_sb[:], 0.0)
            # preload the exp table set during the input DMA wait
            nc.vector.memset(dumm_sb[:], 0.0)
            nc.scalar.activation(out=dumm_sb[:], in_=dumm_sb[:], func=ExpFn)
            # PE heater: warm the HAM clock gate while input DMAs stream
            heat_sb = pers.tile([128, 1024], BF16, tag="heat")
            nc.vector.memset(heat_sb[:], 0.001)
            with tc.tile_pool(name="psH", bufs=1, space="PSUM") as psH:
                hps = psH.tile([128, 512], F32, tag="hps")
                for it in range(32):
                    nc.tensor.matmul(hps[:], lhsT=heat_sb[:, 0:128],
                                     rhs=heat_sb[:, 512:1024],
                                     start=(it == 0), stop=(it == 31))

            # tiny collective issued immediately: the CC stream bootstrap
            # barrier (~45us) and first-op warmup run during early compute
            nc.gpsimd.dma_start(out=dag_in.ap()[:], in_=ones_sb[:, 0:128])
            nc.gpsimd.collective_compute(
                "AllGather", mybir.AluOpType.bypass, replica_groups=GROUPS,
                ins=[dag_in[:]], outs=[dag_out[:]])
            ctxPA = nc.named_scope("phasePA"); ctxPA.__enter__()
            with tc.tile_pool(name="xt", bufs=2) as xtp, \
                 tc.tile_pool(name="ps1", bufs=2, space="PSUM") as ps1p, \
                 tc.tile_pool(name="ex", bufs=3) as exp_pool, \
                 tc.tile_pool(name="sm", bufs=2) as smalls, \
                 tc.tile_pool(name="fnp", bufs=2) as fnp, \
                 tc.tile_pool(name="fatp", bufs=2) as fatp, \
                 tc.tile_pool(name="obp", bufs=2) as obp, \
                 tc.tile_pool(name="psX", bufs=2, space="PSUM") as psX, \
                 tc.tile_pool(name="psA", bufs=2, space="PSUM") as psA:

                xts = {}

                def emit_x_dma(lc):
                    l0 = lc * SEG
                    xtq = xtp.tile([128, NDB, SEG], BF16, tag="xtq",
                                   name=f"xtq_{lc}")
                    xtk = xtp.tile([128, NDB, SEG], BF16, tag="xtk",
                                   name=f"xtk_{lc}")
                    xtv = xtp.tile([128, NDB, SEG], BF16, tag="xtv",
                                   name=f"xtv_{lc}")
                    nc.sync.dma_start(
                        out=xtq[:],
                        in_=xqT.ap().rearrange("(db p) l -> p db l", p=128)
                        [:, :, l0:l0 + SEG])
                    nc.sync.dma_start(
                        out=xtk[:],
                        in_=xkT.ap().rearrange("(db p) l -> p db l", p=128)
                        [:, :, l0:l0 + SEG])
                    nc.scalar.dma_start(
                        out=xtv[:],
                        in_=xvT.ap().rearrange("(db p) l -> p db l", p=128)
                        [:, :, l0:l0 + SEG])
                    xts[lc] = (xtq, xtk, xtv)

                def qk_fillers(lc):
                    """8 half-group fillers projecting q,k for chunk lc.
                    All tile allocations happen inside the closures so pool
                    buffer rotation matches emission order exactly."""
                    l0 = lc * SEG
                    fillers = []
                    for (w_sb, b_sb, t_sb, xi) in ((wq_sb, bq_sb, qT_sb, 0),
                                                   (wk_sb, bk_sb, kT_sb, 1)):
                        cell = {}

                        def h1(cell=cell, w_sb=w_sb, xi=xi, lc=lc):
                            cb = cell["cb"] = cell.get("cb", -1) + 1
                            ps = cell[cb] = ps1p.tile(
                                [128, SEG], F32, tag="ps1",
                                name=f"psqk_{lc}_{xi}_{cb}")
                            x_sb = xts[lc][xi]
                            for db in range(4):
                                nc.tensor.matmul(
                                    ps[:],
                                    lhsT=w_sb[:, db, cb * 128:(cb + 1) * 128],
                                    rhs=x_sb[:, db, :],
                                    start=(db == 0), stop=False)

                        def h2(cell=cell, w_sb=w_sb, b_sb=b_sb, t_sb=t_sb,
                               xi=xi, lc=lc, l0=l0):
                            cb = cell["cb"]
                            ps = cell[cb]
                            x_sb = xts[lc][xi]
                            for db in range(4, NDB):
                                nc.tensor.matmul(
                                    ps[:],
                                    lhsT=w_sb[:, db, cb * 128:(cb + 1) * 128],
                                    rhs=x_sb[:, db, :],
                                    start=False, stop=(db == NDB - 1))
                            nc.vector.tensor_scalar_add(
                                t_sb[:, cb, l0:l0 + SEG], ps[:],
                                b_sb[:, cb:cb + 1])
                        fillers += [h1, h2]
                    # order: q cb0, q cb1, k cb0, k cb1 -> rearrange so both
                    # pair-0 groups (q cb0, k cb0) come first
                    q1, q2, k1, k2 = fillers
                    return [q1, q2, k1, k2, q1, q2, k1, k2]

                def v_fillers(lc):
                    """8 half-group fillers projecting v for chunk lc."""
                    fillers = []
                    for ls in range(4):
                        kbg = lc * 4 + ls
                        cell = {}

                        def h1(cell=cell, ls=ls, lc=lc):
                            psv = cell["psv"] = ps1p.tile(
                                [128, CPC], F32, tag="ps1",
                                name=f"psv_{lc}_{ls}")
                            xtv = xts[lc][2]
                            for db in range(4):
                                nc.tensor.matmul(
                                    psv[:],
                                    lhsT=xtv[:, db, ls * 128:(ls + 1) * 128],
                                    rhs=wv_sb[:, db, :],
                                    start=(db == 0), stop=False)

                        def h2(cell=cell, ls=ls, lc=lc, kbg=kbg):
                            psv = cell["psv"]
                            xtv = xts[lc][2]
                            for db in range(4, NDB):
                                nc.tensor.matmul(
                                    psv[:],
                                    lhsT=xtv[:, db, ls * 128:(ls + 1) * 128],
                                    rhs=wv_sb[:, db, :],
                                    start=False, stop=False)
                            nc.tensor.matmul(
                                psv[:], lhsT=ones_sb[:, 0:128], rhs=bv_sb[:],
                                start=False, stop=True)
                            # padded keys: zero both V rows and the ones col
                            nc.vector.tensor_scalar_mul(
                                v_sb[:, kbg, :, 0:DH],
                                psv[:].rearrange("p (h d) -> p h d", h=HPC),
                                pad_sb[:, kbg:kbg + 1])
                        fillers += [h1, h2]
                    return fillers

                def o_fillers(s):
                    """5 fillers: consume AG chunk s -> out.T rows."""
                    cell = {}

                    def f_dma(cell=cell, s=s):
                        fat = cell["fat"] = fatp.tile(
                            [128, 4, 2, SEG], BF16, tag="fat", name=f"fat_{s}")
                        nc.scalar.dma_start(
                            out=fat[:],
                            in_=ag_out[s].ap().rearrange("r c p l -> c r p l"))
                    fillers = [f_dma]
                    for ob in range(2):
                        def f1(cell=cell, ob=ob, s=s):
                            po = cell[ob] = ps1p.tile(
                                [128, SEG], F32, tag="ps1", name=f"po_{s}_{ob}")
                            fat = cell["fat"]
                            for i, (p, r) in enumerate(
                                    (p, r) for r in range(2) for p in range(2)):
                                nc.tensor.matmul(
                                    po[:],
                                    lhsT=wo_sb[:, 2 * r + p,
                                               ob * 128:(ob + 1) * 128],
                                    rhs=fat[:, r, p, :],
                                    start=(i == 0), stop=False)

                        def f2(cell=cell, ob=ob, s=s):
                            po = cell[ob]
                            fat = cell["fat"]
                            for i, (p, r) in enumerate(
                                    (p, r) for r in range(2, 4) for p in range(2)):
                                nc.tensor.matmul(
                                    po[:],
                                    lhsT=wo_sb[:, 2 * r + p,
                                               ob * 128:(ob + 1) * 128],
                                    rhs=fat[:, r, p, :],
                                    start=False, stop=(i == 3))
                            ob_sb = obp.tile([128, SEG], F32, tag="ob",
                                             name=f"ob_{s}_{ob}")
                            nc.vector.tensor_scalar_add(
                                ob_sb[:], po[:], bo_sb[:, ob:ob + 1])
                            nc.sync.dma_start(
                                out=out[ob * 128:(ob + 1) * 128,
                                        s * SEG:(s + 1) * SEG],
                                in_=ob_sb[:])
                        fillers += [f1, f2]
                    return fillers

                def emit_A_seg(p, s, fillers=(), pre=()):
                    """Attention for pair p, query segment s; returns pa.

                    pre: closures emitted after block 0/1's exp, BEFORE this
                    segment's pa allocation (deferred norm of the previous
                    segment + its AllGather)."""
                    fillers = list(fillers)
                    pre = list(pre)
                    nkb = (s + 1) * 4
                    pa = None
                    for kb in range(nkb):
                        o = max(0, kb * KB - s * SEG)
                        ps = psX.tile([128, 2, SEG], F32, tag="psX",
                                      name=f"ps_{p}_{s}_{kb}")
                        for hp in range(2):
                            hoff = hp * 64
                            # both heads of the pair target different
                            # row groups + PSUM banks -> concurrent MMs
                            nc.tensor.matmul(
                                ps[:, hp, o:SEG],
                                lhsT=kT_sb[hoff:hoff + 64, p,
                                           kb * KB:(kb + 1) * KB],
                                rhs=qT_sb[hoff:hoff + 64, p,
                                          s * SEG + o:(s + 1) * SEG],
                                start=True, stop=True)
                        if kb >= s * 4:  # diagonal block: causal tri
                            for hp in range(2):
                                nc.vector.tensor_add(
                                    ps[:, hp, o:o + 128],
                                    ps[:, hp, o:o + 128], tri_sb[:])
                        ex = exp_pool.tile([128, 2, SEG], BF16, tag="ex",
                                           name=f"ex_{p}_{s}_{kb}")
                        nc.scalar.activation(
                            out=ex[:, :, o:], in_=ps[:, :, o:], func=ExpFn,
                            scale=0.125)
                        if pre:
                            pre.pop(0)()
                        if pa is None:
                            pa = {hp: psA.tile([65, SEG], F32, tag="pa",
                                               name=f"pa_{p}_{s}_{hp}")
                                  for hp in range(2)}
                        for hp in range(2):
                            h = p * 2 + hp
                            nc.tensor.matmul(
                                pa[hp][:, o:], lhsT=v_sb[:, kb, h, :],
                                rhs=ex[:, hp, o:],
                                start=(kb == 0), stop=(kb == nkb - 1))
                        # drain fillers; finish by end of block nkb-2 so
                        # v-projection evacs precede their diagonal blocks
                        nleft = max(1, nkb - 1 - kb)
                        npop = -(-len(fillers) // nleft) if fillers else 0
                        for _ in range(npop):
                            fillers.pop(0)()
                    return pa

                def norm_a(p, s, pa):
                    """Copy S rows off psum, DMA-hop them to partition 0
                    (engines cannot move data across partitions, and the
                    custom reciprocal op only works from partition 0)."""
                    s1 = smalls.tile([65, 2, SEG], F32, tag="s1",
                                     name=f"s1_{p}_{s}")
                    sraw = smalls.tile([1, 2 * SEG], F32, tag="sraw",
                                       name=f"sraw_{p}_{s}")
                    for hp in range(2):
                        nc.vector.tensor_copy(s1[64:65, hp, :],
                                              pa[hp][64:65, :])
                    nc.gpsimd.dma_start(out=sraw[0:1, :],
                                        in_=s1[64:65, :, :])
                    return sraw

                def norm_b(p, s, pa, sraw):
                    """Reciprocal at partition 0, broadcast via one-hot
                    matmul, multiply, ship to the AG bounce."""
                    def run():
                        srec = smalls.tile([1, 2 * SEG], F32, tag="srec",
                                           name=f"srec_{p}_{s}")
                        nc.vector.reciprocal_approx_fast(srec[:], sraw[:])
                        nc.vector.tensor_copy(
                            recb0_sb[0:1, p, :, :],
                            srec[:].rearrange("p (hp n) -> p hp n", hp=2))
                        # K=32 zero-padded one-hot broadcasts 1/S to the 64
                        # channel partitions
                        bcps = psX.tile([128, 2, SEG], F32, tag="psX",
                                        name=f"bc_{p}_{s}")
                        for hp in range(2):
                            nc.tensor.matmul(
                                bcps[0:64, hp, :],
                                lhsT=onesb_sb[0:32, 0:64],
                                rhs=recb0_sb[0:32, p, hp, :],
                                start=True, stop=True)
                        bcs = smalls.tile([64, 2, SEG], F32, tag="bcs",
                                          name=f"bcs_{p}_{s}")
                        nc.vector.tensor_copy(bcs[:], bcps[0:64, :, :])
                        fn = fnp.tile([64, 2, SEG], BF16, tag="fn",
                                      name=f"fn_{p}_{s}")
                        for hp in range(2):
                            nc.vector.tensor_mul(
                                fn[:, hp, :], pa[hp][0:64, :], bcs[:, hp, :])
                        nc.gpsimd.dma_start(
                            out=ag_in[s].ap()[:, p, :].rearrange(
                                "(hp c) l -> c hp l", hp=2),
                            in_=fn[:])
                    return run

                def emit_ag(s):
                    def run():
                        nc.gpsimd.collective_compute(
                            "AllGather", mybir.AluOpType.bypass,
                            replica_groups=GROUPS,
                            ins=[ag_in[s][:]], outs=[ag_out[s][:]])
                    return run

                # ---------------- emission schedule ----------------
                emit_x_dma(0)
                emit_x_dma(1)
                nc.scalar.dma_start(
                    out=wo_sb[:],
                    in_=wo_in.ap().rearrange("(db p) c -> p db c", p=128))
                # chunk 0 projected directly (gates the first exp)
                qk0 = qk_fillers(0)
                vf0 = v_fillers(0)
                for f in qk0[:4]:             # q cb0, k cb0
                    f()
                for f in vf0[:4]:             # v kbg 0,1
                    f()
                pa00 = emit_A_seg(0, 0, vf0[4:])
                s100 = norm_a(0, 0, pa00)
                for f in qk0[4:]:             # q cb1, k cb1
                    f()
                pa10 = emit_A_seg(1, 0, qk_fillers(1),
                                  pre=[norm_b(0, 0, pa00, s100)])
                s110 = norm_a(1, 0, pa10)

                pa01 = emit_A_seg(0, 1, v_fillers(1),
                                  pre=[norm_b(1, 0, pa10, s110), emit_ag(0)])
                s101 = norm_a(0, 1, pa01)
                emit_x_dma(2)
                pa11 = emit_A_seg(1, 1, qk_fillers(2),
                                  pre=[norm_b(0, 1, pa01, s101)])
                s111 = norm_a(1, 1, pa11)

                pa02 = emit_A_seg(0, 2, v_fillers(2),
                                  pre=[norm_b(1, 1, pa11, s111), emit_ag(1)])
                s102 = norm_a(0, 2, pa02)
                emit_x_dma(3)
                pa12 = emit_A_seg(1, 2, qk_fillers(3) + o_fillers(0),
                                  pre=[norm_b(0, 2, pa02, s102)])
                s112 = norm_a(1, 2, pa12)

                pa03 = emit_A_seg(0, 3, v_fillers(3) + o_fillers(1),
                                  pre=[norm_b(1, 2, pa12, s112), emit_ag(2)])
                s103 = norm_a(0, 3, pa03)
                pa13 = emit_A_seg(1, 3, o_fillers(2),
                                  pre=[norm_b(0, 3, pa03, s103)])
                s113 = norm_a(1, 3, pa13)
                norm_b(1, 3, pa13, s113)()
                emit_ag(3)()

                # keep the PE clock warm while AG(3) drains
                hps2 = ps1p.tile([128, 128], F32, tag="ps1", name="hps2")
                for it in range(48):
                    nc.tensor.matmul(hps2[:], lhsT=heat_sb[:, 0:128],
                                     rhs=heat_sb[:, 512:640],
                                     start=(it == 0), stop=(it == 47))
                for f in o_fillers(3):
                    f()

            ctxPA.__exit__(None, None, None)
    nc.compile()
    return nc


def _check_masks(attn_mask, key_padding_mask):
    """The fast path handles exactly-causal attn_mask plus any key padding
    that leaves key 0 unpadded (no all-masked softmax rows)."""
    causal = np.triu(np.ones((L, L), np.bool_), k=1)
    if not np.array_equal(attn_mask, causal):
        return None, True
    if key_padding_mask[:, 0].any():
        return None, True
    pad = [np.ascontiguousarray(
        np.where(key_padding_mask[b].reshape(NKB, KB).T,
                 np.float32(0.0), np.float32(1.0)))
           for b in range(B)]                              # [128, NKB]
    return pad, False


def _host_fallback(query, key, value, attn_mask, key_padding_mask,
                   Wq, bq, Wk, bk, Wv, bv, Wo, bo):
    """Exact fp32 numpy replica of the reference (degenerate masks only)."""
    q = (query @ Wq.T + bq).reshape(B, L, H, DH).transpose(0, 2, 1, 3)
    k = (key @ Wk.T + bk).reshape(B, L, H, DH).transpose(0, 2, 1, 3)
    v = (value @ Wv.T + bv).reshape(B, L, H, DH).transpose(0, 2, 1, 3)
    scores = np.einsum('bhqd,bhkd->bhqk', q, k) / np.sqrt(np.float32(DH))
    scores = np.where(key_padding_mask[:, None, None, :], -1e30, scores)
    scores = np.where(attn_mask[None, None, :, :], -1e30, scores)
    scores = scores - scores.max(axis=-1, keepdims=True)
    w = np.exp(scores)
    w = w / w.sum(axis=-1, keepdims=True)
    attn = np.einsum('bhqk,bhkd->bhqd', w, v)
    attn = attn.transpose(0, 2, 1, 3).reshape(B, L, D)
    return (attn @ Wo.T + bo).astype(np.float32)


def kernel(query, key, value, attn_mask, key_padding_mask,
           Wq, bq, Wk, bk, Wv, bv, Wo, bo):
    global last_results
    query = np.asarray(query, dtype=np.float32)
    key = np.asarray(key, dtype=np.float32)
    value = np.asarray(value, dtype=np.float32)
    attn_mask = np.asarray(attn_mask, dtype=bool)
    key_padding_mask = np.asarray(key_padding_mask, dtype=bool)
    Wq, bq = np.asarray(Wq, np.float32), np.asarray(bq, np.float32)
    Wk, bk = np.asarray(Wk, np.float32), np.asarray(bk, np.float32)
    Wv, bv = np.asarray(Wv, np.float32), np.asarray(bv, np.float32)
    Wo, bo = np.asarray(Wo, np.float32), np.asarray(bo, np.float32)

    pad_bufs, degenerate = _check_masks(attn_mask, key_padding_mask)
    if degenerate:
        return _host_fallback(query, key, value, attn_mask, key_padding_mask,
                              Wq, bq, Wk, bk, Wv, bv, Wo, bo)

    if "prog" not in _PROG_CACHE:
        _PROG_CACHE["prog"] = _build_program()
    nc = _PROG_CACHE["prog"]

    tri = np.where(np.arange(128)[None, :] < np.arange(128)[:, None],
                   np.float32(MASK_VAL), np.float32(0.0))   # [k', q']
    xT_bf = [np.ascontiguousarray(a.transpose(0, 2, 1)).astype(NPBF16)
             for a in (query, key, value)]             # [B, D, L] bf16

    in_maps = []
    for core in range(N_CORES):
        b, j = divmod(core, 4)
        csl = slice(j * CPC, (j + 1) * CPC)
        osl = slice(j * OPC, (j + 1) * OPC)
        vones = np.ascontiguousarray(
            pad_bufs[b][:, :, None, None].repeat(HPC, axis=2)).astype(NPBF16)
        in_maps.append({
            "xqT": xT_bf[0][b],
            "xkT": xT_bf[1][b],
            "xvT": xT_bf[2][b],
            "wqT": np.ascontiguousarray(Wq[csl, :].T).astype(NPBF16),
            "wkT": np.ascontiguousarray(Wk[csl, :].T).astype(NPBF16),
            "wvT": np.ascontiguousarray(Wv[csl, :].T).astype(NPBF16),
            "woT": np.ascontiguousarray(Wo.T[:, osl]).astype(NPBF16),
            "bq": np.ascontiguousarray(bq[csl].reshape(2, 128).T),
            "bk": np.ascontiguousarray(bk[csl].reshape(2, 128).T),
            "bv": bv[csl].reshape(1, CPC).astype(NPBF16),
            "bo2": np.ascontiguousarray(bo[osl].reshape(2, 128).T),
            "pad01": pad_bufs[b],
            "vones": vones,
            "tri": tri,
        })

    trace = os.environ.get("KERNEL_TRACE", "0") == "1"
    res = run_bass_kernel_spmd(nc, in_maps, list(range(N_CORES)), trace=trace)
    last_results = res

    outf = np.empty((B, L, D), dtype=np.float32)
    for core in range(N_CORES):
        b, j = divmod(core, 4)
        outf[b, :, j * OPC:(j + 1) * OPC] = res.results[core]["out"].T
    return outf


# revision 20
# speedup vs baseline: 1.2073x; 1.1670x over previous
"""Distributed MultiHeadAttention kernel for 8 Trainium2 NeuronCores.

Problem: B=2, L=2048, D=1024, H=16 heads (DH=64), causal attn_mask +
key_padding_mask, torch-Linear-convention projections.

Sharding: core = (batch b = core//4, group rank j = core%4). Each core
projects q/k/v for its batch restricted to its 4 heads (256 channels),
runs streaming softmax attention in a [key, query]-transposed layout
(scores are O(1) so no max subtraction; causal masking is structural
per 128-key block plus a -1e5 additive lower-triangle tile on diagonal
blocks). Key padding is folded into V: padded keys get zeroed V rows
AND a zeroed ones-column, so they contribute 0 to both numerator and
denominator -- the exp ACTIVATE needs no per-block bias at all.

v2 restructure vs the 304us baseline:
 - Both head-pairs are processed per query segment, then the segment's
   normalized attention (fp8) ships in ONE small AllGather chunk; 4
   chunked collectives pipeline under attention compute instead of two
   big bf16 AllGathers that exposed a ~57us serial tail.
 - o_proj is sharded by OUTPUT channel (each core computes out.T for
   its 256 output channels over all 2048 rows, host transposes), so
   every core consumes every AG chunk statically the moment it lands;
   o_proj matmuls are interleaved into attention-PE slack as "fillers"
   and replace the PE heater matmuls with real work.
 - Softmax division stays sender-side: S rides row 64 of the AV psum
   (ones column in V), 1/S via fast reciprocal on partition 64, then a
   K=1 matmul broadcasts it across the 64 channel partitions. The
   broadcast/ship half of the norm is deferred into the next segment's
   first block so its DVE dependencies never stall the PE stream.
 - Projection chunks are emitted as fillers inside attention blocks so
   the exp (ACT) stream starts ~12us in and the PE never idles long
   enough to re-throttle (HAM).

Matmuls run in bf16 (fp32 PE matmul is 4x slower); o_proj runs fp8
(weights + gathered activations). Inputs are transposed to [D, L] on
the host (DMA-transpose serializes on the xbar queue; host transpose
is free on the device timeline).
"""
import os
import sys

sys.path.insert(0, '/opt/trn_rl_repo')

import numpy as np
import ml_dtypes

import concourse.bass as bass
import concourse.bacc as bacc
import concourse.mybir as mybir
import concourse.tile as tile
from concourse.bass_utils import run_bass_kernel_spmd

BF16 = mybir.dt.bfloat16
F32 = mybir.dt.float32
FP8 = mybir.dt.float8e4
NPBF16 = ml_dtypes.bfloat16
NPFP8 = ml_dtypes.float8_e4m3

B, L, D, H = 2, 2048, 1024, 16
DH = D // H                      # 64
N_CORES = 8
GROUPS = [[0, 1, 2, 3], [4, 5, 6, 7]]
HPC = H // 4                     # heads per core = 4
CPC = HPC * DH                   # channels per core = 256
OPC = D // 4                     # output channels per core = 256
SEG = 512                        # query segment
NSEG = L // SEG                  # 4
KB = 128                         # key-block size
NKB = L // KB                    # 16
MASK_VAL = -1e5                  # causal: exp(MASK_VAL/8 + s) == 0
NDB = D // 128                   # 8 contraction blocks

ExpFn = mybir.ActivationFunctionType.Exp

_PROG_CACHE = {}
last_results = None


def _build_program():
    """Build the SPMD Bass program (identical on all 8 cores)."""
    nc = bacc.Bacc("TRN2", target_bir_lowering=False, debug=False,
                   num_devices=N_CORES)

    xqT = nc.declare_dram_parameter("xqT", [D, L], BF16, isOutput=False)
    xkT = nc.declare_dram_parameter("xkT", [D, L], BF16, isOutput=False)
    xvT = nc.declare_dram_parameter("xvT", [D, L], BF16, isOutput=False)
    wqT = nc.declare_dram_parameter("wqT", [D, CPC], BF16, isOutput=False)
    wkT = nc.declare_dram_parameter("wkT", [D, CPC], BF16, isOutput=False)
    wvT = nc.declare_dram_parameter("wvT", [D, CPC], BF16, isOutput=False)
    wo_in = nc.declare_dram_parameter("woT", [D, OPC], BF16, isOutput=False)
    bq_in = nc.declare_dram_parameter("bq", [128, 2], F32, isOutput=False)
    bk_in = nc.declare_dram_parameter("bk", [128, 2], F32, isOutput=False)
    bv_in = nc.declare_dram_parameter("bv", [1, CPC], BF16, isOutput=False)
    bo_in = nc.declare_dram_parameter("bo2", [128, 2], F32, isOutput=False)
    pad_in = nc.declare_dram_parameter("pad01", [128, NKB], F32,
                                       isOutput=False)
    vones_in = nc.declare_dram_parameter("vones", [128, NKB, HPC, 1], BF16,
                                         isOutput=False)
    tri_in = nc.declare_dram_parameter("tri", [128, 128], F32, isOutput=False)
    out = nc.declare_dram_parameter("out", [OPC, L], F32, isOutput=True)

    dag_in = nc.dram_tensor("dag_in", [1, 128], BF16)
    dag_out = nc.dram_tensor("dag_out", [4, 1, 128], BF16)
    # per-segment AllGather bounce buffers (fp8 normalized attention):
    # chunk s carries [128ch, pair, 512 q] for query segment s.
    ag_in = [nc.dram_tensor(f"ag_in{s}", [128, 2, SEG], BF16)
             for s in range(NSEG)]
    ag_out = [nc.dram_tensor(f"ag_out{s}", [4, 128, 2, SEG], BF16)
              for s in range(NSEG)]

    with tile.TileContext(nc, num_cores=N_CORES) as tc:
        with tc.tile_pool(name="persist", bufs=1) as pers:
            wq_sb = pers.tile([128, NDB, CPC], BF16, tag="wq")
            wk_sb = pers.tile([128, NDB, CPC], BF16, tag="wk")
            wv_sb = pers.tile([128, NDB, CPC], BF16, tag="wv")
            wo_sb = pers.tile([128, NDB, OPC], BF16, tag="wo")
            bq_sb = pers.tile([128, 2], F32, tag="bq")
            bk_sb = pers.tile([128, 2], F32, tag="bk")
            bv_sb = pers.tile([1, CPC], BF16, tag="bv")
            bo_sb = pers.tile([128, 2], F32, tag="bo")
            pad_sb = pers.tile([128, NKB], F32, tag="pad")
            tri_sb = pers.tile([128, 128], F32, tag="tri")
            ones_sb = pers.tile([1, 128], BF16, tag="ones")
            onesb_sb = pers.tile([128, 64], BF16, tag="onesb")
            qT_sb = pers.tile([128, 2, L], BF16, tag="qT")
            kT_sb = pers.tile([128, 2, L], BF16, tag="kT")
            v_sb = pers.tile([128, NKB, HPC, DH + 1], BF16, tag="v")
            recb0_sb = pers.tile([32, 2, 2, SEG], BF16, tag="recb0")
            dumm_sb = pers.tile([1, 1], BF16, tag="dumm")

            # spread big loads across three DMA queues; small constants go
            # on the (idle-early) gpsimd queue so they never delay x-chunks
            nc.sync.dma_start(
                out=wq_sb[:], in_=wqT.ap().rearrange("(db p) c -> p db c", p=128))
            nc.scalar.dma_start(
                out=wk_sb[:], in_=wkT.ap().rearrange("(db p) c -> p db c", p=128))
            nc.scalar.dma_start(
                out=wv_sb[:], in_=wvT.ap().rearrange("(db p) c -> p db c", p=128))
            nc.gpsimd.dma_start(out=bq_sb[:], in_=bq_in[:])
            nc.gpsimd.dma_start(out=bk_sb[:], in_=bk_in[:])
            nc.gpsimd.dma_start(out=bv_sb[:], in_=bv_in[:])
            nc.gpsimd.dma_start(out=bo_sb[:], in_=bo_in[:])
            nc.gpsimd.dma_start(out=pad_sb[:], in_=pad_in[:])
            nc.gpsimd.dma_start(out=tri_sb[:], in_=tri_in[:])
            nc.gpsimd.dma_start(out=v_sb[:, :, :, DH:DH + 1], in_=vones_in[:])
            nc.vector.memset(ones_sb[:], 1.0)
            # row 0 = 1, rows 1-31 = 0: the broadcast matmul runs K=32
            # from partition 0 because HW contracts the full 32-row group
            # (stale weights x garbage otherwise; the sim won't show this)
            nc.vector.memset(onesb_sb[:], 0.0)
            nc.vector.memset(onesb_sb[0:1, :], 1.0)
            # 1/S staging rows; recb0 rows 1-31 stay zero forever so the
            # K=32 broadcast contracts zeros beyond row 0
            nc.vector.memset(recb0_sb[:], 0.0)
            # preload the exp table set during the input DMA wait
            nc.vector.memset(dumm_sb[:], 0.0)
            nc.scalar.activation(out=dumm_sb[:], in_=dumm_sb[:], func=ExpFn)
            # PE heater: warm the HAM clock gate while input DMAs stream
            heat_sb = pers.tile([128, 1024], BF16, tag="heat")
            nc.vector.memset(heat_sb[:], 0.001)
            with tc.tile_pool(name="psH", bufs=1, space="PSUM") as psH:
                hps = psH.tile([128, 512], F32, tag="hps")
                for it in range(32):
                    nc.tensor.matmul(hps[:], lhsT=heat_sb[:, 0:128],
                                     rhs=heat_sb[:, 512:1024],
                                     start=(it == 0), stop=(it == 31))

            # tiny collective issued immediately: the CC stream bootstrap
            # barrier (~45us) and first-op warmup run during early compute
            nc.gpsimd.dma_start(out=dag_in.ap()[:], in_=ones_sb[:, 0:128])
            nc.gpsimd.collective_compute(
                "AllGather", mybir.AluOpType.bypass, replica_groups=GROUPS,
                ins=[dag_in[:]], outs=[dag_out[:]])
            ctxPA = nc.named_scope("phasePA"); ctxPA.__enter__()
            with tc.tile_pool(name="xt", bufs=2) as xtp, \
                 tc.tile_pool(name="ps1", bufs=2, space="PSUM") as ps1p, \
                 tc.tile_pool(name="ex", bufs=3) as exp_pool, \
                 tc.tile_pool(name="sm", bufs=2) as smalls, \
                 tc.tile_pool(name="fnp", bufs=2) as fnp, \
                 tc.tile_pool(name="fatp", bufs=2) as fatp, \
                 tc.tile_pool(name="obp", bufs=2) as obp, \
                 tc.tile_pool(name="psX", bufs=2, space="PSUM") as psX, \
                 tc.tile_pool(name="psA", bufs=2, space="PSUM") as psA:

                xts = {}

                def emit_x_dma(lc):
                    l0 = lc * SEG
                    xtq = xtp.tile([128, NDB, SEG], BF16, tag="xtq",
                                   name=f"xtq_{lc}")
                    xtk = xtp.tile([128, NDB, SEG], BF16, tag="xtk",
                                   name=f"xtk_{lc}")
                    xtv = xtp.tile([128, NDB, SEG], BF16, tag="xtv",
                                   name=f"xtv_{lc}")
                    nc.sync.dma_start(
                        out=xtq[:],
                        in_=xqT.ap().rearrange("(db p) l -> p db l", p=128)
                        [:, :, l0:l0 + SEG])
                    nc.sync.dma_start(
                        out=xtk[:],
                        in_=xkT.ap().rearrange("(db p) l -> p db l", p=128)
                        [:, :, l0:l0 + SEG])
                    nc.sync.dma_start(
                        out=xtv[:],
                        in_=xvT.ap().rearrange("(db p) l -> p db l", p=128)
                        [:, :, l0:l0 + SEG])
                    xts[lc] = (xtq, xtk, xtv)

                def qk_fillers(lc):
                    """8 half-group fillers projecting q,k for chunk lc.
                    All tile allocations happen inside the closures so pool
                    buffer rotation matches emission order exactly."""
                    l0 = lc * SEG
                    fillers = []
                    for (w_sb, b_sb, t_sb, xi) in ((wq_sb, bq_sb, qT_sb, 0),
                                                   (wk_sb, bk_sb, kT_sb, 1)):
                        cell = {}

                        def h1(cell=cell, w_sb=w_sb, xi=xi, lc=lc):
                            cb = cell["cb"] = cell.get("cb", -1) + 1
                            ps = cell[cb] = ps1p.tile(
                                [128, SEG], F32, tag="ps1",
                                name=f"psqk_{lc}_{xi}_{cb}")
                            x_sb = xts[lc][xi]
                            for db in range(4):
                                nc.tensor.matmul(
                                    ps[:],
                                    lhsT=w_sb[:, db, cb * 128:(cb + 1) * 128],
                                    rhs=x_sb[:, db, :],
                                    start=(db == 0), stop=False)

                        def h2(cell=cell, w_sb=w_sb, b_sb=b_sb, t_sb=t_sb,
                               xi=xi, lc=lc, l0=l0):
                            cb = cell["cb"]
                            ps = cell[cb]
                            x_sb = xts[lc][xi]
                            for db in range(4, NDB):
                                nc.tensor.matmul(
                                    ps[:],
                                    lhsT=w_sb[:, db, cb * 128:(cb + 1) * 128],
                                    rhs=x_sb[:, db, :],
                                    start=False, stop=(db == NDB - 1))
                            nc.vector.tensor_scalar_add(
                                t_sb[:, cb, l0:l0 + SEG], ps[:],
                                b_sb[:, cb:cb + 1])
                        fillers += [h1, h2]
                    # order: q cb0, q cb1, k cb0, k cb1 -> rearrange so both
                    # pair-0 groups (q cb0, k cb0) come first
                    q1, q2, k1, k2 = fillers
                    return [q1, q2, k1, k2, q1, q2, k1, k2]

                def v_fillers(lc):
                    """8 half-group fillers projecting v for chunk lc."""
                    fillers = []
                    for ls in range(4):
                        kbg = lc * 4 + ls
                        cell = {}

                        def h1(cell=cell, ls=ls, lc=lc):
                            psv = cell["psv"] = ps1p.tile(
                                [128, CPC], F32, tag="ps1",
                                name=f"psv_{lc}_{ls}")
                            xtv = xts[lc][2]
                            for db in range(4):
                                nc.tensor.matmul(
                                    psv[:],
                                    lhsT=xtv[:, db, ls * 128:(ls + 1) * 128],
                                    rhs=wv_sb[:, db, :],
                                    start=(db == 0), stop=False)

                        def h2(cell=cell, ls=ls, lc=lc, kbg=kbg):
                            psv = cell["psv"]
                            xtv = xts[lc][2]
                            for db in range(4, NDB):
                                nc.tensor.matmul(
                                    psv[:],
                                    lhsT=xtv[:, db, ls * 128:(ls + 1) * 128],
                                    rhs=wv_sb[:, db, :],
                                    start=False, stop=False)
                            nc.tensor.matmul(
                                psv[:], lhsT=ones_sb[:, 0:128], rhs=bv_sb[:],
                                start=False, stop=True)
                            # padded keys: zero both V rows and the ones col
                            nc.vector.tensor_scalar_mul(
                                v_sb[:, kbg, :, 0:DH],
                                psv[:].rearrange("p (h d) -> p h d", h=HPC),
                                pad_sb[:, kbg:kbg + 1])
                        fillers += [h1, h2]
                    return fillers

                def o_fillers(s):
                    """5 fillers: consume AG chunk s -> out.T rows."""
                    cell = {}

                    def f_dma(cell=cell, s=s):
                        fat = cell["fat"] = fatp.tile(
                            [128, 4, 2, SEG], BF16, tag="fat", name=f"fat_{s}")
                        nc.sync.dma_start(
                            out=fat[:],
                            in_=ag_out[s].ap().rearrange("r c p l -> c r p l"))
                    fillers = [f_dma]
                    for ob in range(2):
                        def f1(cell=cell, ob=ob, s=s):
                            po = cell[ob] = ps1p.tile(
                                [128, SEG], F32, tag="ps1", name=f"po_{s}_{ob}")
                            fat = cell["fat"]
                            for i, (p, r) in enumerate(
                                    (p, r) for r in range(2) for p in range(2)):
                                nc.tensor.matmul(
                                    po[:],
                                    lhsT=wo_sb[:, 2 * r + p,
                                               ob * 128:(ob + 1) * 128],
                                    rhs=fat[:, r, p, :],
                                    start=(i == 0), stop=False)

                        def f2(cell=cell, ob=ob, s=s):
                            po = cell[ob]
                            fat = cell["fat"]
                            for i, (p, r) in enumerate(
                                    (p, r) for r in range(2, 4) for p in range(2)):
                                nc.tensor.matmul(
                                    po[:],
                                    lhsT=wo_sb[:, 2 * r + p,
                                               ob * 128:(ob + 1) * 128],
                                    rhs=fat[:, r, p, :],
                                    start=False, stop=(i == 3))
                            ob_sb = obp.tile([128, SEG], F32, tag="ob",
                                             name=f"ob_{s}_{ob}")
                            nc.vector.tensor_scalar_add(
                                ob_sb[:], po[:], bo_sb[:, ob:ob + 1])
                            nc.sync.dma_start(
                                out=out[ob * 128:(ob + 1) * 128,
                                        s * SEG:(s + 1) * SEG],
                                in_=ob_sb[:])
                        fillers += [f1, f2]
                    return fillers

                def emit_A_seg(p, s, fillers=(), pre=()):
                    """Attention for pair p, query segment s; returns pa.

                    pre: closures emitted after block 0/1's exp, BEFORE this
                    segment's pa allocation (deferred norm of the previous
                    segment + its AllGather)."""
                    fillers = list(fillers)
                    pre = list(pre)
                    nkb = (s + 1) * 4
                    pa = None
                    for kb in range(nkb):
                        o = max(0, kb * KB - s * SEG)
                        ps = psX.tile([128, 2, SEG], F32, tag="psX",
                                      name=f"ps_{p}_{s}_{kb}")
                        for hp in range(2):
                            hoff = hp * 64
                            # both heads of the pair target different
                            # row groups + PSUM banks -> concurrent MMs
                            nc.tensor.matmul(
                                ps[:, hp, o:SEG],
                                lhsT=kT_sb[hoff:hoff + 64, p,
                                           kb * KB:(kb + 1) * KB],
                                rhs=qT_sb[hoff:hoff + 64, p,
                                          s * SEG + o:(s + 1) * SEG],
                                start=True, stop=True)
                        if kb >= s * 4:  # diagonal block: causal tri
                            for hp in range(2):
                                nc.vector.tensor_add(
                                    ps[:, hp, o:o + 128],
                                    ps[:, hp, o:o + 128], tri_sb[:])
                        ex = exp_pool.tile([128, 2, SEG], BF16, tag="ex",
                                           name=f"ex_{p}_{s}_{kb}")
                        nc.scalar.activation(
                            out=ex[:, :, o:], in_=ps[:, :, o:], func=ExpFn,
                            scale=0.125)
                        if pre:
                            pre.pop(0)()
                        if pa is None:
                            pa = {hp: psA.tile([65, SEG], F32, tag="pa",
                                               name=f"pa_{p}_{s}_{hp}")
                                  for hp in range(2)}
                        for hp in range(2):
                            h = p * 2 + hp
                            nc.tensor.matmul(
                                pa[hp][:, o:], lhsT=v_sb[:, kb, h, :],
                                rhs=ex[:, hp, o:],
                                start=(kb == 0), stop=(kb == nkb - 1))
                        # drain fillers; finish by end of block nkb-2 so
                        # v-projection evacs precede their diagonal blocks
                        nleft = max(1, nkb - 1 - kb)
                        npop = -(-len(fillers) // nleft) if fillers else 0
                        for _ in range(npop):
                            fillers.pop(0)()
                    return pa

                def norm_a(p, s, pa):
                    """Copy S rows off psum, DMA-hop them to partition 0
                    (engines cannot move data across partitions, and the
                    custom reciprocal op only works from partition 0)."""
                    s1 = smalls.tile([65, 2, SEG], F32, tag="s1",
                                     name=f"s1_{p}_{s}")
                    sraw = smalls.tile([1, 2 * SEG], F32, tag="sraw",
                                       name=f"sraw_{p}_{s}")
                    for hp in range(2):
                        nc.vector.tensor_copy(s1[64:65, hp, :],
                                              pa[hp][64:65, :])
                    nc.sync.dma_start(out=sraw[0:1, :],
                                       in_=s1[64:65, :, :])
                    return sraw

                def norm_b(p, s, pa, sraw):
                    """Reciprocal at partition 0, broadcast via one-hot
                    matmul, multiply, ship to the AG bounce."""
                    def run():
                        srec = smalls.tile([1, 2 * SEG], F32, tag="srec",
                                           name=f"srec_{p}_{s}")
                        nc.vector.reciprocal_approx_fast(srec[:], sraw[:])
                        nc.vector.tensor_copy(
                            recb0_sb[0:1, p, :, :],
                            srec[:].rearrange("p (hp n) -> p hp n", hp=2))
                        # K=32 zero-padded one-hot broadcasts 1/S to the 64
                        # channel partitions
                        bcps = psX.tile([128, 2, SEG], F32, tag="psX",
                                        name=f"bc_{p}_{s}")
                        for hp in range(2):
                            nc.tensor.matmul(
                                bcps[0:64, hp, :],
                                lhsT=onesb_sb[0:32, 0:64],
                                rhs=recb0_sb[0:32, p, hp, :],
                                start=True, stop=True)
                        bcs = smalls.tile([64, 2, SEG], F32, tag="bcs",
                                          name=f"bcs_{p}_{s}")
                        nc.vector.tensor_copy(bcs[:], bcps[0:64, :, :])
                        fn = fnp.tile([64, 2, SEG], BF16, tag="fn",
                                      name=f"fn_{p}_{s}")
                        for hp in range(2):
                            nc.vector.tensor_mul(
                                fn[:, hp, :], pa[hp][0:64, :], bcs[:, hp, :])
                        nc.sync.dma_start(
                            out=ag_in[s].ap()[:, p, :].rearrange(
                                "(hp c) l -> c hp l", hp=2),
                            in_=fn[:])
                    return run

                def emit_ag(s):
                    def run():
                        nc.gpsimd.collective_compute(
                            "AllGather", mybir.AluOpType.bypass,
                            replica_groups=GROUPS,
                            ins=[ag_in[s][:]], outs=[ag_out[s][:]])
                    return run

                # ---------------- emission schedule ----------------
                emit_x_dma(0)
                emit_x_dma(1)
                nc.scalar.dma_start(
                    out=wo_sb[:],
                    in_=wo_in.ap().rearrange("(db p) c -> p db c", p=128))
                # chunk 0 projected directly (gates the first exp)
                qk0 = qk_fillers(0)
                vf0 = v_fillers(0)
                for f in qk0[:4]:             # q cb0, k cb0
                    f()
                for f in vf0[:4]:             # v kbg 0,1
                    f()
                pa00 = emit_A_seg(0, 0, vf0[4:])
                s100 = norm_a(0, 0, pa00)
                for f in qk0[4:]:             # q cb1, k cb1
                    f()
                pa10 = emit_A_seg(1, 0, qk_fillers(1),
                                  pre=[norm_b(0, 0, pa00, s100)])
                s110 = norm_a(1, 0, pa10)

                pa01 = emit_A_seg(0, 1, v_fillers(1),
                                  pre=[norm_b(1, 0, pa10, s110), emit_ag(0)])
                s101 = norm_a(0, 1, pa01)
                emit_x_dma(2)
                pa11 = emit_A_seg(1, 1, qk_fillers(2),
                                  pre=[norm_b(0, 1, pa01, s101)])
                s111 = norm_a(1, 1, pa11)

                pa02 = emit_A_seg(0, 2, v_fillers(2),
                                  pre=[norm_b(1, 1, pa11, s111), emit_ag(1)])
                s102 = norm_a(0, 2, pa02)
                emit_x_dma(3)
                pa12 = emit_A_seg(1, 2, qk_fillers(3) + o_fillers(0),
                                  pre=[norm_b(0, 2, pa02, s102)])
                s112 = norm_a(1, 2, pa12)

                pa03 = emit_A_seg(0, 3, v_fillers(3) + o_fillers(1),
                                  pre=[norm_b(1, 2, pa12, s112), emit_ag(2)])
                s103 = norm_a(0, 3, pa03)
                pa13 = emit_A_seg(1, 3, o_fillers(2),
                                  pre=[norm_b(0, 3, pa03, s103)])
                s113 = norm_a(1, 3, pa13)
                norm_b(1, 3, pa13, s113)()
                emit_ag(3)()

                # keep the PE clock warm while AG(3) drains
                hps2 = ps1p.tile([128, 128], F32, tag="ps1", name="hps2")
                for it in range(48):
                    nc.tensor.matmul(hps2[:], lhsT=heat_sb[:, 0:128],
                                     rhs=heat_sb[:, 512:640],
                                     start=(it == 0), stop=(it == 47))
                for f in o_fillers(3):
                    f()

            ctxPA.__exit__(None, None, None)
    nc.compile()
    return nc


def _check_masks(attn_mask, key_padding_mask):
    """The fast path handles exactly-causal attn_mask plus any key padding
    that leaves key 0 unpadded (no all-masked softmax rows)."""
    causal = np.triu(np.ones((L, L), np.bool_), k=1)
    if not np.array_equal(attn_mask, causal):
        return None, True
    if key_padding_mask[:, 0].any():
        return None, True
    pad = [np.ascontiguousarray(
        np.where(key_padding_mask[b].reshape(NKB, KB).T,
                 np.float32(0.0), np.float32(1.0)))
           for b in range(B)]                              # [128, NKB]
    return pad, False


def _host_fallback(query, key, value, attn_mask, key_padding_mask,
                   Wq, bq, Wk, bk, Wv, bv, Wo, bo):
    """Exact fp32 numpy replica of the reference (degenerate masks only)."""
    q = (query @ Wq.T + bq).reshape(B, L, H, DH).transpose(0, 2, 1, 3)
    k = (key @ Wk.T + bk).reshape(B, L, H, DH).transpose(0, 2, 1, 3)
    v = (value @ Wv.T + bv).reshape(B, L, H, DH).transpose(0, 2, 1, 3)
    scores = np.einsum('bhqd,bhkd->bhqk', q, k) / np.sqrt(np.float32(DH))
    scores = np.where(key_padding_mask[:, None, None, :], -1e30, scores)
    scores = np.where(attn_mask[None, None, :, :], -1e30, scores)
    scores = scores - scores.max(axis=-1, keepdims=True)
    w = np.exp(scores)
    w = w / w.sum(axis=-1, keepdims=True)
    attn = np.einsum('bhqk,bhkd->bhqd', w, v)
    attn = attn.transpose(0, 2, 1, 3).reshape(B, L, D)
    return (attn @ Wo.T + bo).astype(np.float32)


def kernel(query, key, value, attn_mask, key_padding_mask,
           Wq, bq, Wk, bk, Wv, bv, Wo, bo):
    global last_results
    query = np.asarray(query, dtype=np.float32)
    key = np.asarray(key, dtype=np.float32)
    value = np.asarray(value, dtype=np.float32)
    attn_mask = np.asarray(attn_mask, dtype=bool)
    key_padding_mask = np.asarray(key_padding_mask, dtype=bool)
    Wq, bq = np.asarray(Wq, np.float32), np.asarray(bq, np.float32)
    Wk, bk = np.asarray(Wk, np.float32), np.asarray(bk, np.float32)
    Wv, bv = np.asarray(Wv, np.float32), np.asarray(bv, np.float32)
    Wo, bo = np.asarray(Wo, np.float32), np.asarray(bo, np.float32)

    pad_bufs, degenerate = _check_masks(attn_mask, key_padding_mask)
    if degenerate:
        return _host_fallback(query, key, value, attn_mask, key_padding_mask,
                              Wq, bq, Wk, bk, Wv, bv, Wo, bo)

    if "prog" not in _PROG_CACHE:
        _PROG_CACHE["prog"] = _build_program()
    nc = _PROG_CACHE["prog"]

    tri = np.where(np.arange(128)[None, :] < np.arange(128)[:, None],
                   np.float32(MASK_VAL), np.float32(0.0))   # [k', q']
    xT_bf = [np.ascontiguousarray(a.transpose(0, 2, 1)).astype(NPBF16)
             for a in (query, key, value)]             # [B, D, L] bf16

    in_maps = []
    for core in range(N_CORES):
        b, j = divmod(core, 4)
        csl = slice(j * CPC, (j + 1) * CPC)
        osl = slice(j * OPC, (j + 1) * OPC)
        vones = np.ascontiguousarray(
            pad_bufs[b][:, :, None, None].repeat(HPC, axis=2)).astype(NPBF16)
        in_maps.append({
            "xqT": xT_bf[0][b],
            "xkT": xT_bf[1][b],
            "xvT": xT_bf[2][b],
            "wqT": np.ascontiguousarray(Wq[csl, :].T).astype(NPBF16),
            "wkT": np.ascontiguousarray(Wk[csl, :].T).astype(NPBF16),
            "wvT": np.ascontiguousarray(Wv[csl, :].T).astype(NPBF16),
            "woT": np.ascontiguousarray(Wo.T[:, osl]).astype(NPBF16),
            "bq": np.ascontiguousarray(bq[csl].reshape(2, 128).T),
            "bk": np.ascontiguousarray(bk[csl].reshape(2, 128).T),
            "bv": bv[csl].reshape(1, CPC).astype(NPBF16),
            "bo2": np.ascontiguousarray(bo[osl].reshape(2, 128).T),
            "pad01": pad_bufs[b],
            "vones": vones,
            "tri": tri,
        })

    trace = os.environ.get("KERNEL_TRACE", "0") == "1"
    res = run_bass_kernel_spmd(nc, in_maps, list(range(N_CORES)), trace=trace)
    last_results = res

    outf = np.empty((B, L, D), dtype=np.float32)
    for core in range(N_CORES):
        b, j = divmod(core, 4)
        outf[b, :, j * OPC:(j + 1) * OPC] = res.results[core]["out"].T
    return outf
